# revision 1
# baseline (speedup 1.0000x reference)
"""Nystrom attention Trainium2 kernel.

Sharding: 8 cores = 4 batches x 2 head-groups (4 heads each).
Each core computes its (batch, head-group) slice end-to-end, including its
share of the output projection; the host sums the two partial output
projections per batch and adds bo.

Device layouts (per core, one SPMD program):
  xT   [512, 8192]  bf16   x[b] transposed (emb on partitions)
  wqk  [512, 512]   bf16   [Wq_heads*scale | Wk_heads*scale]
  wv   [512, 256]   bf16
  wo   [256, 512]   bf16   Wo rows for this head group
  bqk  [512]        f32    [bq_heads | bk_heads]*scale
  bvb  [128, 256]   f32    bv broadcast to 128 partitions
  idbf [128, 128]   bf16   identity (PE transpose)
  idf32[128, 128]   f32    identity
  nsc  [64, 192]    f32    [7*I | 15*I | 3.25*I]
  onesr[1, 128]     f32    ones row
Output:
  out  [8192, 512]  f32    partial (x@..@Wo for this head group), no bo
"""

import numpy as np
import ml_dtypes

import concourse.bass as bass
import concourse.tile as tile
from concourse import bacc, mybir
from concourse.bass_utils import run_bass_kernel_spmd

BF16 = mybir.dt.bfloat16
F32 = mybir.dt.float32
AF = mybir.ActivationFunctionType
AX = mybir.AxisListType
OP = mybir.AluOpType

S = 8192        # sequence length
E = 512         # embedding dim
D = 64          # head dim
L = 64          # landmarks
NHG = 4         # heads per core (head group)
N_ITER = 6
SCALE = 1.0 / np.sqrt(np.sqrt(D))

_CACHED_NC = None


def _build(phases=(1, 2, 3)):
    nc = bacc.Bacc("TRN2", target_bir_lowering=False, debug=False, num_devices=8)

    xT_d = nc.dram_tensor("xT", [E, S], BF16, kind="ExternalInput").ap()
    wqk_d = nc.dram_tensor("wqk", [E, 512], BF16, kind="ExternalInput").ap()
    wv_d = nc.dram_tensor("wv", [E, 256], BF16, kind="ExternalInput").ap()
    wo_d = nc.dram_tensor("wo", [256, E], BF16, kind="ExternalInput").ap()
    bqk_d = nc.dram_tensor("bqk", [512], F32, kind="ExternalInput").ap()
    bvb_d = nc.dram_tensor("bvb", [128, 256], F32, kind="ExternalInput").ap()
    idbf_d = nc.dram_tensor("idbf", [128, 128], BF16, kind="ExternalInput").ap()
    idf32_d = nc.dram_tensor("idf32", [128, 128], F32, kind="ExternalInput").ap()
    nsc_d = nc.dram_tensor("nsc", [64, 192], F32, kind="ExternalInput").ap()
    onesr_d = nc.dram_tensor("onesr", [1, 128], F32, kind="ExternalInput").ap()
    blk1_d = nc.dram_tensor("blk1", [128, 128], BF16,
                            kind="ExternalInput").ap()
    out_d = nc.dram_tensor("out", [S, E], F32, kind="ExternalOutput").ap()

    with tile.TileContext(nc) as tc:
        _emit(nc, tc, xT_d, wqk_d, wv_d, wo_d, bqk_d, bvb_d, idbf_d, idf32_d,
              nsc_d, onesr_d, blk1_d, out_d, phases)
    nc.compile()
    return nc


def _emit(nc, tc, xT_d, wqk_d, wv_d, wo_d, bqk_d, bvb_d, idbf_d, idf32_d,
          nsc_d, onesr_d, blk1_d, out_d, phases=(1, 2, 3)):
    with (
        tc.tile_pool(name="const", bufs=1) as const,
        tc.tile_pool(name="big", bufs=1) as big,
        tc.tile_pool(name="small", bufs=2) as small,
    ):
        # ---- constants / weights into SBUF ----
        wqk_sb = const.tile([128, 4, 512], BF16, tag="wqk")
        nc.sync.dma_start(wqk_sb[:], wqk_d.rearrange("(ko p) m -> p ko m", p=128))
        wv_sb = const.tile([128, 4, 256], BF16, tag="wv")
        nc.sync.dma_start(wv_sb[:], wv_d.rearrange("(ko p) m -> p ko m", p=128))
        wo_sb = const.tile([128, 2, 512], BF16, tag="wo")
        nc.sync.dma_start(wo_sb[:], wo_d.rearrange("(j p) m -> p j m", p=128))
        bqk_sb = const.tile([128, 4], F32, tag="bqk")
        nc.sync.dma_start(bqk_sb[:], bqk_d.rearrange("(t p) -> p t", p=128))
        bvb_sb = const.tile([128, 256], F32, tag="bvb")
        nc.sync.dma_start(bvb_sb[:], bvb_d[:])
        idbf_sb = const.tile([128, 128], BF16, tag="idbf")
        nc.sync.dma_start(idbf_sb[:], idbf_d[:])
        idf32_sb = const.tile([128, 128], F32, tag="idf32")
        nc.sync.dma_start(idf32_sb[:], idf32_d[:])
        nsc_sb = const.tile([64, 192], F32, tag="nsc")
        nc.sync.dma_start(nsc_sb[:], nsc_d[:])
        onesr_sb = const.tile([1, 128], F32, tag="onesr")
        nc.sync.dma_start(onesr_sb[:], onesr_d[:])
        blk1_sb = const.tile([128, 128], BF16, tag="blk1")
        nc.sync.dma_start(blk1_sb[:], blk1_d[:])

        # ---- persistent activations ----
        qT = big.tile([128, 2, S], BF16, tag="qT")      # (2h d | seq), per hp
        kT = big.tile([128, 2, S], BF16, tag="kT")
        vsb = big.tile([128, 64, 4, 65], BF16, tag="v")  # (s | chunk, head, d+1)
        landq = const.tile([128, 2, L], F32, tag="landq")  # raw segment sums
        landk = const.tile([128, 2, L], F32, tag="landk")

        nc.vector.memset(vsb[:, :, :, 64:65], 1.0)

        xT_t = xT_d.rearrange("(ko p) s -> p ko s", p=128)

        # ================= Phase 1: QKV projection =================
        if 1 not in phases:
            pass
        else:
         with (
            tc.tile_pool(name="xt", bufs=4) as xpool,
            tc.tile_pool(name="ps_qk", bufs=5, space="PSUM") as ps_qk,
            tc.tile_pool(name="ps_v", bufs=3, space="PSUM") as ps_v,
        ):
            for c in range(16):
                sl = bass.ts(c, 512)
                xt = xpool.tile([128, 4, 512], BF16, tag="xt")
                nc.sync.dma_start(xt[:], xT_t[:, :, sl])
                for t in range(4):  # q01 q23 k01 k23
                    ps = ps_qk.tile([128, 512], F32, tag="psqk")
                    for ko in range(4):
                        nc.tensor.matmul(
                            ps[:], lhsT=wqk_sb[:, ko, bass.ts(t, 128)],
                            rhs=xt[:, ko, :], start=(ko == 0), stop=(ko == 3))
                    dst = qT if t < 2 else kT
                    hp = t % 2
                    nc.scalar.activation(dst[:, hp, sl], ps[:], AF.Identity,
                                         bias=bqk_sb[:, t:t + 1])
                    land = landq if t < 2 else landk
                    nc.vector.reduce_sum(
                        land[:, hp, bass.ts(c, 4)],
                        dst[:, hp, sl].rearrange("p (g s) -> p g s", s=128),
                        axis=AX.X)
                for s4 in range(4):
                    psv = ps_v.tile([128, 256], F32, tag="psv")
                    for ko in range(4):
                        nc.tensor.matmul(
                            psv[:], lhsT=xt[:, ko, bass.ts(s4, 128)],
                            rhs=wv_sb[:, ko, :], start=(ko == 0), stop=(ko == 3))
                    nc.vector.tensor_tensor(
                        vsb[:, c * 4 + s4, :, 0:64],
                        psv[:].rearrange("p (h d) -> p h d", d=64),
                        bvb_sb[:].rearrange("p (h d) -> p h d", d=64),
                        op=OP.add)

        # ---- landmark means (+bias), cast bf16, build block-diagonals ----
        landq_bf = const.tile([128, 2, L], BF16, tag="landqbf")
        landk_bf = const.tile([128, 2, L], BF16, tag="landkbf")
        # qT/kT already carry the bias, so only the 1/seg scale here
        for hp in range(2):
            nc.vector.tensor_scalar_mul(landq_bf[:, hp, :], landq[:, hp, :],
                                        1.0 / 128.0)
            nc.vector.tensor_scalar_mul(landk_bf[:, hp, :], landk[:, hp, :],
                                        1.0 / 128.0)
        qblk = []
        kblk = []
        for hp in range(2):
            qb = const.tile([128, 128], BF16, tag=f"qblk{hp}")
            kb = const.tile([128, 128], BF16, tag=f"kblk{hp}")
            for b_ in (qb, kb):
                nc.vector.memset(b_[:], 0.0)
            nc.vector.tensor_copy(qb[0:64, 0:64], landq_bf[0:64, hp, :])
            nc.vector.tensor_copy(qb[64:128, 64:128], landq_bf[64:128, hp, :])
            nc.vector.tensor_copy(kb[0:64, 0:64], landk_bf[0:64, hp, :])
            nc.vector.tensor_copy(kb[64:128, 64:128], landk_bf[64:128, hp, :])
            qblk.append(qb)
            kblk.append(kb)

        # block-diagonal t2 (lhsT of the hcT matmul), built later
        t2blk = []
        for hp in range(2):
            tb = const.tile([128, 128], BF16, tag=f"t2blk{hp}")
            nc.vector.memset(tb[:], 0.0)
            t2blk.append(tb)

        # ============ Phase 2a: kernel_3 -> t1 accumulation ============
        K2 = const.tile([128, L], F32, tag="K2")  # both heads of hp stacked
        K2s = [K2]
        K2b = const.tile([128, L], F32, tag="K2b")
        K2s.append(K2b)
        if 2 in phases:
         with tc.tile_pool(name="ps_t1", bufs=4, space="PSUM") as ps_t1:
            t1ps = [ps_t1.tile([65, 64], F32, tag="t1", name=f"t1ps{i}")
                    for i in range(4)]
            with (
                tc.tile_pool(name="ps_s3", bufs=4, space="PSUM") as ps_s3,
                tc.tile_pool(name="e3p", bufs=3) as e3p,
            ):
                for cg in range(16):
                    for hp in range(2):
                        ps3 = ps_s3.tile([128, 512], F32, tag="ps3")
                        for i in range(4):
                            nc.tensor.matmul(
                                ps3[:, bass.ts(i, 128)],
                                lhsT=kT[:, hp, bass.ts(cg * 4 + i, 128)],
                                rhs=qblk[hp], start=True, stop=True)
                        e3 = e3p.tile([128, 512], BF16, tag="e3")
                        nc.scalar.activation(e3[:], ps3[:], AF.Exp)
                        for i in range(4):
                            c = cg * 4 + i
                            for h2 in range(2):
                                h = hp * 2 + h2
                                nc.tensor.matmul(
                                    t1ps[h][:], lhsT=vsb[:, c, h, :],
                                    rhs=e3[:, i * 128 + h2 * 64:
                                           i * 128 + h2 * 64 + 64],
                                    start=(c == 0), stop=(c == 63),
                                    skip_group_check=True)
                # kernel_2 (landmark x landmark) while ps_s3 still open
                for hp in range(2):
                    ps2 = ps_s3.tile([128, 128], F32, tag="ps3")
                    nc.tensor.matmul(ps2[:], lhsT=qblk[hp], rhs=kblk[hp],
                                     start=True, stop=True)
                    k2e = small.tile([128, L], F32, tag="k2e")
                    rs = small.tile([128, 1], F32, tag="k2rs")
                    for h2 in range(2):
                        nc.scalar.activation(k2e[bass.ts(h2, 64), :],
                                             ps2[bass.ts(h2, 64), bass.ts(h2, 64)],
                                             AF.Exp,
                                             accum_out=rs[bass.ts(h2, 64), :])
                    ri = small.tile([128, 1], F32, tag="k2ri")
                    nc.vector.reciprocal(ri[:], rs[:])
                    nc.vector.tensor_scalar_mul(K2s[hp][:], k2e[:], ri[:])

            # ========= Phase 2b: Newton-Schulz inverse + t2 =========
            with (
                tc.tile_pool(name="ps_ns", bufs=4, space="PSUM") as ps_ns,
                tc.tile_pool(name="nsp", bufs=2) as nsp,
            ):
                id64 = idf32_sb[0:64, 0:64]
                HS = [(h, h // 2, h % 2, bass.ts(h % 2, 64)) for h in range(4)]
                K2T = {}
                mxi = {}
                V = {}
                W = {}
                for h, hp, h2, psl in HS:
                    K2h = K2s[hp][psl, :]
                    pk = ps_ns.tile([65, 65], F32, tag="ns", name=f"pk{h}")
                    nc.tensor.transpose(pk[0:64, 0:64], K2h,
                                        idf32_sb[psl, psl])
                    K2T[h] = nsp.tile([64, 64], F32, tag=f"K2T{h}",
                                      name=f"K2T{h}")
                    nc.vector.tensor_copy(K2T[h][:], pk[0:64, 0:64])
                for h, hp, h2, psl in HS:
                    # max column-sum of K2  (= max row-sum of K2T)
                    cs = nsp.tile([64, 1], F32, tag=f"cs{h}", name=f"cs{h}")
                    nc.vector.reduce_sum(cs[:], K2T[h][:], axis=AX.X)
                    pc = ps_ns.tile([65, 65], F32, tag="ns", name=f"pc{h}")
                    nc.tensor.transpose(pc[0:1, 0:64], cs[:], id64)
                    mx = nsp.tile([1, 1], F32, tag=f"mx{h}", name=f"mx{h}")
                    nc.vector.reduce_max(mx[:], pc[0:1, 0:64], axis=AX.X)
                    pb = ps_ns.tile([65, 65], F32, tag="ns", name=f"pb{h}")
                    nc.tensor.matmul(pb[0:64, 0:1], lhsT=onesr_sb[0:1, 0:64],
                                     rhs=mx[:], start=True, stop=True)
                    mxi[h] = nsp.tile([64, 1], F32, tag=f"mxi{h}",
                                      name=f"mxi{h}")
                    nc.vector.reciprocal(mxi[h][:], pb[0:64, 0:1])
                for h, hp, h2, psl in HS:
                    V[h] = nsp.tile([64, 64], F32, tag=f"V{h}", name=f"V{h}")
                    nc.vector.tensor_scalar_mul(V[h][:], K2T[h][:], mxi[h][:])
                    W[h] = nsp.tile([64, 64], F32, tag=f"W{h}", name=f"W{h}")
                    nc.vector.tensor_scalar_mul(W[h][:], K2s[hp][psl, :],
                                                mxi[h][:])

                for _ in range(N_ITER):
                    pkv = {}
                    T1 = {}
                    KVT = {}
                    T2 = {}
                    T3 = {}
                    for h, hp, h2, psl in HS:
                        p = ps_ns.tile([65, 65], F32, tag="ns", name=f"pkv{h}")
                        nc.tensor.matmul(p[0:64, 0:64], lhsT=K2T[h][:],
                                         rhs=V[h][:], start=True, stop=True)
                        pkv[h] = p
                    for h, hp, h2, psl in HS:
                        T1[h] = nsp.tile([64, 64], F32, tag=f"T1{h}",
                                         name=f"T1{h}")
                        nc.vector.tensor_tensor(T1[h][:], nsc_sb[:, 0:64],
                                                pkv[h][0:64, 0:64],
                                                op=OP.subtract)
                        p = ps_ns.tile([65, 65], F32, tag="ns", name=f"pvt{h}")
                        nc.tensor.matmul(p[0:64, 0:64], lhsT=V[h][:],
                                         rhs=K2T[h][:], start=True, stop=True)
                        KVT[h] = nsp.tile([64, 64], F32, tag=f"KVT{h}",
                                          name=f"KVT{h}")
                        nc.vector.tensor_copy(KVT[h][:], p[0:64, 0:64])
                    for h, hp, h2, psl in HS:
                        p = ps_ns.tile([65, 65], F32, tag="ns", name=f"p3{h}")
                        nc.tensor.matmul(p[0:64, 0:64], lhsT=KVT[h][:],
                                         rhs=T1[h][:], start=True, stop=True)
                        T2[h] = nsp.tile([64, 64], F32, tag=f"T2{h}",
                                         name=f"T2{h}")
                        nc.vector.tensor_tensor(T2[h][:], nsc_sb[:, 64:128],
                                                p[0:64, 0:64], op=OP.subtract)
                    for h, hp, h2, psl in HS:
                        p = ps_ns.tile([65, 65], F32, tag="ns", name=f"p4{h}")
                        nc.tensor.matmul(p[0:64, 0:64], lhsT=KVT[h][:],
                                         rhs=T2[h][:], start=True, stop=True)
                        T3[h] = nsp.tile([64, 64], F32, tag=f"T3{h}",
                                         name=f"T3{h}")
                        nc.vector.scalar_tensor_tensor(
                            T3[h][:], p[0:64, 0:64], -0.25,
                            nsc_sb[:, 128:192], op0=OP.mult, op1=OP.add)
                    for h, hp, h2, psl in HS:
                        p5 = ps_ns.tile([65, 65], F32, tag="ns", name=f"p5{h}")
                        nc.tensor.matmul(p5[0:64, 0:64], lhsT=W[h][:],
                                         rhs=T3[h][:], start=True, stop=True)
                        p6 = ps_ns.tile([65, 65], F32, tag="ns", name=f"p6{h}")
                        nc.tensor.matmul(p6[0:64, 0:64], lhsT=T3[h][:],
                                         rhs=W[h][:], start=True, stop=True)
                        V[h] = nsp.tile([64, 64], F32, tag=f"V{h}",
                                        name=f"V{h}")
                        nc.vector.tensor_copy(V[h][:], p5[0:64, 0:64])
                        W[h] = nsp.tile([64, 64], F32, tag=f"W{h}",
                                        name=f"W{h}")
                        nc.vector.tensor_copy(W[h][:], p6[0:64, 0:64])

                t1n = {}
                for h, hp, h2, psl in HS:
                    # t1 normalize: transpose [65,64] -> [64,65]
                    t1u = nsp.tile([65, 64], F32, tag=f"t1u{h}", name=f"t1u{h}")
                    nc.vector.tensor_copy(t1u[:], t1ps[h][:])
                    ptt = ps_ns.tile([65, 65], F32, tag="ns", name=f"ptt{h}")
                    nc.tensor.transpose(ptt[0:64, 0:65], t1u[:],
                                        idf32_sb[0:65, 0:65])
                    d3i = nsp.tile([64, 1], F32, tag=f"d3i{h}", name=f"d3i{h}")
                    nc.vector.reciprocal(d3i[:], ptt[0:64, 64:65])
                    t1n[h] = nsp.tile([64, 64], F32, tag=f"t1n{h}",
                                      name=f"t1n{h}")
                    nc.vector.tensor_scalar_mul(t1n[h][:], ptt[0:64, 0:64],
                                                d3i[:])
                for h, hp, h2, psl in HS:
                    # t2 = V2 @ t1n  (lhsT = W = V2^T)
                    pt2 = ps_ns.tile([65, 65], F32, tag="ns", name=f"pt2{h}")
                    nc.tensor.matmul(pt2[0:64, 0:64], lhsT=W[h][:],
                                     rhs=t1n[h][:], start=True, stop=True)
                    nc.vector.tensor_copy(t2blk[hp][psl, psl],
                                          pt2[0:64, 0:64])

        # ======= Phase 3: kernel_1, apply, output projection =======
        if 3 not in phases:
            return
        with (
            tc.tile_pool(name="ps_s1", bufs=2, space="PSUM") as ps_s1,
            tc.tile_pool(name="ps_rb", bufs=2, space="PSUM") as ps_rb,
            tc.tile_pool(name="ps_ht", bufs=2, space="PSUM") as ps_ht,
            tc.tile_pool(name="ps_out", bufs=2, space="PSUM") as ps_out,
            tc.tile_pool(name="e1p", bufs=3) as e1p,
            tc.tile_pool(name="hcp", bufs=3) as hcp,
        ):
            for c in range(16):
                hcts = []
                for hp in range(2):
                    ps1 = ps_s1.tile([128, 512], F32, tag="ps1")
                    nc.tensor.matmul(ps1[:], lhsT=kblk[hp],
                                     rhs=qT[:, hp, bass.ts(c, 512)],
                                     start=True, stop=True)
                    e1t = e1p.tile([128, 512], BF16, tag="e1")
                    nc.scalar.activation(e1t[:], ps1[:], AF.Exp)
                    e1 = e1t[:]
                    # per-head kernel_1 row-sums, pre-broadcast to the
                    # (head, d) partition layout via block-ones matmul
                    prb = ps_rb.tile([128, 512], F32, tag="prb")
                    nc.tensor.matmul(prb[:], lhsT=blk1_sb[:], rhs=e1[:],
                                     start=True, stop=True)
                    rbs = e1p.tile([128, 512], F32, tag="rbs")
                    nc.vector.reciprocal(rbs[:], prb[:])
                    # hcT (unnormalized) = blockdiag(t2).T @ e1 -> [(h,d), s]
                    pht = ps_ht.tile([128, 512], F32, tag="pht")
                    nc.tensor.matmul(pht[:], lhsT=t2blk[hp], rhs=e1[:],
                                     start=True, stop=True)
                    hct = hcp.tile([128, 512], BF16, tag="hct")
                    nc.vector.tensor_tensor(hct[:], pht[:], rbs[:],
                                            op=OP.mult)
                    hcts.append(hct)
                for s4 in range(4):
                    c128 = c * 4 + s4
                    pso2 = ps_out.tile([128, 512], F32, tag="psout")
                    for hp in range(2):
                        nc.tensor.matmul(pso2[:],
                                         lhsT=hcts[hp][:, bass.ts(s4, 128)],
                                         rhs=wo_sb[:, hp, :],
                                         start=(hp == 0), stop=(hp == 1))
                    osb = hcp.tile([128, 512], F32, tag="osb")
                    nc.scalar.copy(osb[:], pso2[:])
                    nc.sync.dma_start(out_d[bass.ts(c128, 128), :], osb[:])


def _prep_inputs(x, Wq, bq, Wk, bk, Wv, bv, Wo, bo):
    bf = ml_dtypes.bfloat16
    x = np.asarray(x, dtype=np.float32)
    Wq = np.asarray(Wq, dtype=np.float32)
    Wk = np.asarray(Wk, dtype=np.float32)
    Wv = np.asarray(Wv, dtype=np.float32)
    Wo = np.asarray(Wo, dtype=np.float32)
    bq = np.asarray(bq, dtype=np.float32)
    bk = np.asarray(bk, dtype=np.float32)
    bv = np.asarray(bv, dtype=np.float32)

    idf = np.eye(128, dtype=np.float32)
    consts = {
        "idbf": np.ascontiguousarray(idf.astype(bf)),
        "idf32": idf,
        "nsc": np.ascontiguousarray(np.concatenate(
            [7.0 * np.eye(64), 15.0 * np.eye(64), 3.25 * np.eye(64)],
            axis=1).astype(np.float32)),
        "onesr": np.ones((1, 128), dtype=np.float32),
        "blk1": np.ascontiguousarray(
            np.kron(np.eye(2), np.ones((64, 64))).astype(bf)),
    }
    in_maps = []
    for core in range(8):
        b, g = core // 2, core % 2
        hsl = slice(g * 256, (g + 1) * 256)
        xT = np.ascontiguousarray(x[b].T.astype(bf))
        wqk = np.ascontiguousarray(
            np.concatenate([Wq[:, hsl], Wk[:, hsl]], axis=1) * SCALE).astype(bf)
        wv = np.ascontiguousarray(Wv[:, hsl]).astype(bf)
        wo = np.ascontiguousarray(Wo[hsl, :]).astype(bf)
        bqk = np.ascontiguousarray(
            np.concatenate([bq[hsl], bk[hsl]]) * SCALE).astype(np.float32)
        bvb = np.ascontiguousarray(
            np.broadcast_to(bv[hsl], (128, 256))).astype(np.float32)
        in_maps.append({
            "xT": xT, "wqk": wqk, "wv": wv, "wo": wo,
            "bqk": bqk, "bvb": bvb, **consts,
        })
    return in_maps


def run_on_device(in_maps, **kwargs):
    global _CACHED_NC
    if _CACHED_NC is None:
        _CACHED_NC = _build()
    return run_bass_kernel_spmd(_CACHED_NC, in_maps, core_ids=list(range(8)),
                                **kwargs)


def kernel(x, Wq, bq, Wk, bk, Wv, bv, Wo, bo):
    in_maps = _prep_inputs(x, Wq, bq, Wk, bk, Wv, bv, Wo, bo)
    res = run_on_device(in_maps)
    bo = np.asarray(bo, dtype=np.float32)
    out = np.empty((4, S, E), dtype=np.float32)
    for b in range(4):
        out[b] = res.results[2 * b]["out"] + res.results[2 * b + 1]["out"] + bo
    return out

